# revision 16
# baseline (speedup 1.0000x reference)
"""MultiHeadSelfAttention Trainium2 Bass kernel.

Shapes (hardcoded): B=8, N=2048, E=512, H=8 heads, D=64 head dim.
Sharding: data-parallel over batch -> one batch item per NeuronCore (8 cores),
no collectives needed.

Per-core pipeline (bf16 compute, fp32 accumulate):
  stage 0: load Q/K/V/W f32, cast bf16 (GpSimd), PE-transpose into X^T
           layouts; v = V @ Wv^T augmented with a per-head ones column
           (v_aug) so P@V also yields softmax row sums; proj for head-pair 0.
  attention (per head, q-chunk of 512, key-tile pair): S^T [128 keys, 2x512]
           in PSUM = k^T . q^T; Exp on ScalarE (1/tau folded into scale);
           P^T bf16; O^T [65, 512] += v_aug^T . P^T accumulated over 16 key
           tiles; row 64 = softmax denominator. Projections for the next
           head-pair are interleaved into this stream as PE filler, plus an
           optional duplicated scores matmul, to keep the tensor engine
           saturated (the HAM clock gate drops PE to 1.2 GHz whenever the
           PE stream has persistent micro-gaps).
  tail: reciprocal of row sums, DMA-broadcast, normalize O^T; Y^T = Wo @
        O^T + bo; PE-transpose back to [2048, 512], cast fp32, DMA out.

The attention mask is all ones per the problem spec; validated host-side.
"""

import sys

for _p in ("/opt/trn_rl_repo",):
    if _p not in sys.path:
        sys.path.insert(0, _p)

import numpy as np
from collections import deque
from contextlib import ExitStack

import concourse.bass as bass
import concourse.bacc as bacc
import concourse.mybir as mybir
import concourse.tile as tile
from concourse.masks import make_identity

B, N, E = 8, 2048, 512
H, D = 8, 64
P = 128          # partitions
ET = E // P      # 4 e-tiles
NT = N // P      # 16 n-tiles
QC = 512         # q chunk in attention
NQC = N // QC    # 4
KTP = NT // 2    # 8 key-tile pairs
HV = 65          # head dim + ones column
FP32 = mybir.dt.float32
BF16 = mybir.dt.bfloat16
NCORES = 8
WARM_DUP = 0     # extra duplicated scores matmuls per ktp (PE warmth filler)

AF = mybir.ActivationFunctionType
ALU = mybir.AluOpType


def _build(inv_tau: float) -> bass.Bass:
    nc = bacc.Bacc(trn_type="TRN2")

    dQ = nc.dram_tensor("Q", [N, E], FP32, kind="ExternalInput")
    dK = nc.dram_tensor("K", [N, E], FP32, kind="ExternalInput")
    dV = nc.dram_tensor("V", [N, E], FP32, kind="ExternalInput")
    dWq = nc.dram_tensor("Wq", [E, E], FP32, kind="ExternalInput")
    dWk = nc.dram_tensor("Wk", [E, E], FP32, kind="ExternalInput")
    dWv = nc.dram_tensor("Wv", [E, E], FP32, kind="ExternalInput")
    dWo = nc.dram_tensor("Wo", [E, E], FP32, kind="ExternalInput")
    dbo = nc.dram_tensor("bo", [E], FP32, kind="ExternalInput")
    dout = nc.dram_tensor("out", [N, E], FP32, kind="ExternalOutput")
    drs = nc.dram_tensor("r_scratch", [H * N], FP32)

    with tile.TileContext(nc) as tc, ExitStack() as ctx:
        _body(ctx, tc, inv_tau, dQ, dK, dV, dWq, dWk, dWv, dWo, dbo, dout, drs)
    nc.finalize()
    return nc


def _body(ctx, tc, inv_tau, dQ, dK, dV, dWq, dWk, dWv, dWo, dbo, dout, drs):
    nc = tc.nc
    dma = nc.sync.dma_start

    const = ctx.enter_context(tc.tile_pool(name="const", bufs=1))
    # 12 x [128, N] bf16 slots reused across phases:
    #   stage 0: Q^T (big_0..3) / K^T (big_4..7) / V^T (big_8..11)
    #   attn+tail: oT (big_8..11, V dead), oTn (big_0..3, Q dead),
    #              yT (big_4..7, K dead)
    big = ctx.enter_context(tc.tile_pool(name="big", bufs=1))
    proj = ctx.enter_context(tc.tile_pool(name="proj", bufs=1))
    # one PSUM pool for the whole kernel; 8 banks total:
    #   tp (transpose staging, f32-sized, bufs=2)       -> 2 banks
    #   pp (proj/outproj accum, f32 [128,512], bufs=1)  -> 1 bank
    #   s2 (scores, f32 [128,1024], bufs=2)             -> 4 banks
    #   o2 (PV accum, f32 [65,512], bufs=1)             -> 1 bank
    psum = ctx.enter_context(tc.tile_pool(name="psum", bufs=1, space="PSUM"))
    stage = ctx.enter_context(tc.tile_pool(name="stage", bufs=4))
    p2pool = ctx.enter_context(tc.tile_pool(name="p2pool", bufs=2))

    ident = const.tile([P, P], BF16, name="ident", tag="ident")
    make_identity(nc, ident)

    bo_sb = const.tile([P, ET], FP32, name="bo_sb", tag="bo_sb")
    dma(out=bo_sb, in_=dbo[:].rearrange("(t p) -> p t", p=P))

    l1 = const.tile([1, H * N], FP32, name="l1", tag="l1")

    def load_cast(dX, r):
        """DMA [128,E] f32 slice r, cast to bf16 on gpsimd."""
        x_f32 = stage.tile([P, E], FP32, name="x_f32", tag="x_f32", bufs=3)
        dma(out=x_f32, in_=dX[r * P:(r + 1) * P, :])
        x_bf = stage.tile([P, E], BF16, name="x_bf", tag="x_bf", bufs=5)
        nc.vector.tensor_copy(x_bf, x_f32)
        return x_bf

    # ---- stage 0: weights ----
    wt = {}
    for wname, _ in (("q", dWq), ("k", dWk), ("v", dWv), ("o", dWo)):
        wt[wname] = [const.tile([P, E], BF16, name=f"w{wname}T_{c}",
                                tag=f"w{wname}T_{c}") for c in range(ET)]
    for wname, dW in (("q", dWq), ("k", dWk), ("v", dWv), ("o", dWo)):
        w_bfs = [load_cast(dW, r) for r in range(ET)]
        for c in range(ET):
            tp = psum.tile([P, E], BF16, name="tp", tag="tp", bufs=2)
            for r in range(ET):
                nc.tensor.transpose(
                    tp[:, r * P:(r + 1) * P], w_bfs[r][:, c * P:(c + 1) * P],
                    ident)
            nc.vector.tensor_copy(wt[wname][c], tp)

    # ---- stage 0: inputs -> X^T ----
    xT = {}
    slot = {"Q": 0, "K": 4, "V": 8}
    for xname, dX in (("Q", dQ), ("K", dK), ("V", dV)):
        xT[xname] = [big.tile([P, N], BF16, name=f"{xname}T_{et}",
                              tag=f"big_{slot[xname] + et}")
                     for et in range(ET)]
        for g in range(NT // ET):  # groups of 4 n-tiles
            x_bfs = [load_cast(dX, g * ET + i) for i in range(ET)]
            for et in range(ET):
                tp = psum.tile([P, E], BF16, name="tp", tag="tp", bufs=2)
                for i in range(ET):
                    nc.tensor.transpose(
                        tp[:, i * P:(i + 1) * P],
                        x_bfs[i][:, et * P:(et + 1) * P], ident)
                nc.vector.tensor_copy(
                    xT[xname][et][:, g * E:(g + 1) * E], tp)

    # ---- projections ----
    qT = [proj.tile([P, N], BF16, name=f"qT_{m}", tag=f"qT_{m}")
          for m in range(ET)]
    kT = [proj.tile([P, N], BF16, name=f"kT_{m}", tag=f"kT_{m}")
          for m in range(ET)]
    v_aug = [proj.tile([P, H * HV], BF16, name=f"vaug_{nt}",
                       tag=f"vaug_{nt}") for nt in range(NT)]

    def emit_qk_proj(m, c):
        """qT[m] and kT[m], n-chunk c: two 4-MM groups + drains."""
        for pname, outs, xtiles in (("q", qT, xT["Q"]), ("k", kT, xT["K"])):
            ps = psum.tile([P, 512], FP32, name="pp", tag="pp", bufs=1)
            for et in range(ET):
                nc.tensor.matmul(
                    ps,
                    lhsT=wt[pname][et][:, m * P:(m + 1) * P],
                    rhs=xtiles[et][:, c * 512:(c + 1) * 512],
                    start=(et == 0), stop=(et == ET - 1))
            nc.vector.tensor_copy(outs[m][:, c * 512:(c + 1) * 512], ps)

    def emit_v_proj(nt):
        ps = psum.tile([P, 512], FP32, name="pp", tag="pp", bufs=1)
        for et in range(ET):
            nc.tensor.matmul(
                ps,
                lhsT=xT["V"][et][:, nt * P:(nt + 1) * P],
                rhs=wt["v"][et],
                start=(et == 0), stop=(et == ET - 1))
        va = v_aug[nt].rearrange("p (h c) -> p h c", c=HV)
        nc.vector.tensor_copy(
            va[:, :, 0:D], ps.rearrange("p (h d) -> p h d", d=D))
        nc.gpsimd.memset(va[:, :, D:HV], 1.0)

    # v fully + head-pair 0 projections up front
    for nt in range(NT):
        emit_v_proj(nt)
    for c in range(ET):
        emit_qk_proj(0, c)

    # ---- attention, with interleaved proj filler ----
    oT = [big.tile([P, N], BF16, name=f"oT_{m}", tag=f"big_{8 + m}")
          for m in range(ET)]

    filler = deque()
    for hp in range(1, ET):
        for c in range(ET):
            filler.append((hp, c))

    def emit_pv(h, o2, p2, ktp):
        for j in range(2):
            kt = 2 * ktp + j
            nc.tensor.matmul(
                o2[:, :],
                lhsT=v_aug[kt][:, h * HV:(h + 1) * HV],
                rhs=p2[:, j * QC:(j + 1) * QC],
                start=(kt == 0), stop=(kt == NT - 1),
                skip_group_check=True)

    def attn_chunk(h, qc):
        hp, h2 = h // 2, (h % 2) * 64
        o2 = psum.tile([HV, QC], FP32, name="o2", tag="o2", bufs=1)
        rhs_q = qT[hp][h2:h2 + 64, qc * QC:(qc + 1) * QC]
        prev = None  # (p2, ktp) pending PV
        for ktp in range(KTP):
            s2 = psum.tile([P, 2 * QC], FP32, name="s2", tag="s2", bufs=2)
            for j in range(2):
                kt = 2 * ktp + j
                lhs_k = kT[hp][h2:h2 + 64, kt * P:(kt + 1) * P]
                nc.tensor.matmul(s2[:, j * QC:(j + 1) * QC], lhsT=lhs_k,
                                 rhs=rhs_q, start=True, stop=True)
            for _ in range(WARM_DUP):
                gb = psum.tile([P, QC], FP32, name="gb", tag="tp", bufs=2)
                nc.tensor.matmul(gb, lhsT=kT[hp][h2:h2 + 64, 0:P],
                                 rhs=rhs_q, start=True, stop=True)
            if prev is not None:
                emit_pv(h, o2, *prev)
            p2 = p2pool.tile([P, 2 * QC], BF16, name="p2", tag="p2")
            nc.scalar.activation(p2, s2, AF.Exp, scale=inv_tau)
            prev = (p2, ktp)
            if ktp % 2 == 1 and filler:
                emit_qk_proj(*filler.popleft())
        emit_pv(h, o2, *prev)
        nc.vector.tensor_copy(
            l1[0:1, h * N + qc * QC:h * N + (qc + 1) * QC], o2[D:HV, :])
        nc.vector.tensor_copy(
            oT[hp][h2:h2 + 64, qc * QC:(qc + 1) * QC], o2[0:D, :])

    oTn = [big.tile([P, N], BF16, name=f"oTn_{m}", tag=f"big_{m}")
           for m in range(ET)]

    def emit_norm(hp):
        lsl = l1[0:1, 2 * hp * N:2 * (hp + 1) * N]
        nc.vector.reciprocal(lsl, lsl)
        dma(out=drs[2 * hp * N:2 * (hp + 1) * N], in_=lsl)
        for qc in range(NQC):
            rb = stage.tile([P, QC], FP32, name="rb", tag="rb", bufs=2)
            for h2 in range(2):
                h = hp * 2 + h2
                bsrc = bass.AP(
                    tensor=drs,
                    offset=h * N + qc * QC,
                    ap=[[0, 64], [1, QC]])
                dma(out=rb[h2 * 64:(h2 + 1) * 64, :], in_=bsrc)
            nc.vector.tensor_tensor(
                oTn[hp][:, qc * QC:(qc + 1) * QC],
                oT[hp][:, qc * QC:(qc + 1) * QC],
                rb, ALU.mult)

    for h in range(H):
        for qc in range(NQC):
            attn_chunk(h, qc)
        if h % 2 == 1:
            emit_norm(h // 2)

    # ---- output projection: Y^T = Wo @ O^T + bo ----
    yT = [big.tile([P, N], BF16, name=f"yT_{m}", tag=f"big_{4 + m}")
          for m in range(ET)]
    for m in range(ET):
        for c in range(ET):
            ps = psum.tile([P, 512], FP32, name="pp", tag="pp", bufs=1)
            for et in range(ET):
                nc.tensor.matmul(
                    ps,
                    lhsT=wt["o"][et][:, m * P:(m + 1) * P],
                    rhs=oTn[et][:, c * 512:(c + 1) * 512],
                    start=(et == 0), stop=(et == ET - 1))
            nc.scalar.add(yT[m][:, c * 512:(c + 1) * 512], ps,
                          bo_sb[:, m:m + 1])

    # ---- transpose back + store ----
    for nt in range(NT):
        tp = psum.tile([P, E], BF16, name="tp", tag="tp", bufs=2)
        for m in range(ET):
            nc.tensor.transpose(
                tp[:, m * P:(m + 1) * P],
                yT[m][:, nt * P:(nt + 1) * P], ident)
        y_sb = stage.tile([P, E], FP32, name="y_sb", tag="y_sb", bufs=2)
        nc.scalar.copy(y_sb, tp)
        dma(out=dout[nt * P:(nt + 1) * P, :], in_=y_sb)


_CACHE = {}


def _get_nc(inv_tau: float) -> bass.Bass:
    key = round(float(inv_tau), 9)
    if key not in _CACHE:
        _CACHE[key] = _build(float(inv_tau))
    return _CACHE[key]


def _run(inputs: dict, trace: bool = False):
    """Returns (output [B,N,E] fp32, BassKernelResults)."""
    from concourse.bass_utils import run_bass_kernel_spmd

    Q = np.ascontiguousarray(np.asarray(inputs["Q"], dtype=np.float32))
    K = np.ascontiguousarray(np.asarray(inputs["K"], dtype=np.float32))
    V = np.ascontiguousarray(np.asarray(inputs["V"], dtype=np.float32))
    Wq = np.ascontiguousarray(np.asarray(inputs["Wq"], dtype=np.float32))
    Wk = np.ascontiguousarray(np.asarray(inputs["Wk"], dtype=np.float32))
    Wv = np.ascontiguousarray(np.asarray(inputs["Wv"], dtype=np.float32))
    Wo = np.ascontiguousarray(np.asarray(inputs["Wo"], dtype=np.float32))
    bo = np.ascontiguousarray(np.asarray(inputs["bo"], dtype=np.float32))
    tau = float(np.asarray(inputs["tau"]))

    mask = inputs.get("attn_mask")
    if mask is not None and not np.all(np.asarray(mask) != 0):
        # Fallback (never hit for the spec'd all-ones mask): host math.
        return _host_reference(Q, K, V, np.asarray(mask), Wq, Wk, Wv, Wo,
                               bo, tau), None

    nc = _get_nc(1.0 / tau)
    in_maps = []
    for b in range(NCORES):
        in_maps.append({
            "Q": Q[b], "K": K[b], "V": V[b],
            "Wq": Wq, "Wk": Wk, "Wv": Wv, "Wo": Wo, "bo": bo,
        })
    res = run_bass_kernel_spmd(nc, in_maps, list(range(NCORES)), trace=trace)
    out = np.stack([np.asarray(res.results[b]["out"]) for b in range(NCORES)])
    return out.astype(np.float32), res


def _host_reference(Q, K, V, mask, Wq, Wk, Wv, Wo, bo, tau):
    b, n, _ = Q.shape
    q = (Q @ Wq.T).reshape(b, n, H, D).transpose(0, 2, 1, 3)
    k = (K @ Wk.T).reshape(b, n, H, D).transpose(0, 2, 1, 3)
    v = (V @ Wv.T).reshape(b, n, H, D).transpose(0, 2, 1, 3)
    s = np.einsum("bhnd,bhmd->bhnm", q, k) / tau
    s = np.where(mask == 0, -np.inf, s)
    s = s - s.max(axis=-1, keepdims=True)
    e = np.exp(s)
    a = e / e.sum(axis=-1, keepdims=True)
    o = np.einsum("bhnm,bhmd->bhnd", a, v)
    o = o.transpose(0, 2, 1, 3).reshape(b, n, H * D)
    return (o @ Wo.T + bo).astype(np.float32)


def kernel(**inputs) -> np.ndarray:
    out, _ = _run(inputs, trace=False)
    return out


# revision 17
# speedup vs baseline: 1.2950x; 1.2950x over previous
"""MultiHeadSelfAttention Trainium2 Bass kernel.

Shapes (hardcoded): B=8, N=2048, E=512, H=8 heads, D=64 head dim.
Sharding: data-parallel over batch -> one batch item per NeuronCore (8 cores),
no collectives needed.

Per-core pipeline (bf16 compute, fp32 accumulate):
  stage 0: load Q/K/V/W f32, cast bf16 (GpSimd), PE-transpose into X^T
           layouts; v = V @ Wv^T augmented with a per-head ones column
           (v_aug) so P@V also yields softmax row sums; proj for head-pair 0.
  attention (per head, q-chunk of 512, key-tile pair): S^T [128 keys, 2x512]
           in PSUM = k^T . q^T; Exp on ScalarE (1/tau folded into scale);
           P^T bf16; O^T [65, 512] += v_aug^T . P^T accumulated over 16 key
           tiles; row 64 = softmax denominator. Projections for the next
           head-pair are interleaved into this stream as PE filler, plus an
           optional duplicated scores matmul, to keep the tensor engine
           saturated (the HAM clock gate drops PE to 1.2 GHz whenever the
           PE stream has persistent micro-gaps).
  tail: reciprocal of row sums, DMA-broadcast, normalize O^T; Y^T = Wo @
        O^T + bo; PE-transpose back to [2048, 512], cast fp32, DMA out.

The attention mask is all ones per the problem spec; validated host-side.
"""

import sys

for _p in ("/opt/trn_rl_repo",):
    if _p not in sys.path:
        sys.path.insert(0, _p)

import numpy as np
from collections import deque
from contextlib import ExitStack

import concourse.bass as bass
import concourse.bacc as bacc
import concourse.mybir as mybir
import concourse.tile as tile
from concourse.masks import make_identity

B, N, E = 8, 2048, 512
H, D = 8, 64
P = 128          # partitions
ET = E // P      # 4 e-tiles
NT = N // P      # 16 n-tiles
QC = 512         # q chunk in attention
NQC = N // QC    # 4
KTP = NT // 2    # 8 key-tile pairs
HV = 65          # head dim + ones column
FP32 = mybir.dt.float32
BF16 = mybir.dt.bfloat16
NCORES = 8
WARM_DUP = 0     # extra duplicated scores matmuls per ktp (PE warmth filler)

AF = mybir.ActivationFunctionType
ALU = mybir.AluOpType


def _build(inv_tau: float) -> bass.Bass:
    nc = bacc.Bacc(trn_type="TRN2")

    dQ = nc.dram_tensor("Q", [N, E], FP32, kind="ExternalInput")
    dK = nc.dram_tensor("K", [N, E], FP32, kind="ExternalInput")
    dV = nc.dram_tensor("V", [N, E], FP32, kind="ExternalInput")
    dWq = nc.dram_tensor("Wq", [E, E], FP32, kind="ExternalInput")
    dWk = nc.dram_tensor("Wk", [E, E], FP32, kind="ExternalInput")
    dWv = nc.dram_tensor("Wv", [E, E], FP32, kind="ExternalInput")
    dWo = nc.dram_tensor("Wo", [E, E], FP32, kind="ExternalInput")
    dbo = nc.dram_tensor("bo", [E], FP32, kind="ExternalInput")
    dout = nc.dram_tensor("out", [N, E], FP32, kind="ExternalOutput")
    drs = nc.dram_tensor("r_scratch", [H * N], FP32)

    with tile.TileContext(nc) as tc, ExitStack() as ctx:
        _body(ctx, tc, inv_tau, dQ, dK, dV, dWq, dWk, dWv, dWo, dbo, dout, drs)
    nc.finalize()
    return nc


def _body(ctx, tc, inv_tau, dQ, dK, dV, dWq, dWk, dWv, dWo, dbo, dout, drs):
    nc = tc.nc
    dma = nc.sync.dma_start

    const = ctx.enter_context(tc.tile_pool(name="const", bufs=1))
    # 12 x [128, N] bf16 slots reused across phases:
    #   stage 0: Q^T (big_0..3) / K^T (big_4..7) / V^T (big_8..11)
    #   attn+tail: oT (big_8..11, V dead), oTn (big_0..3, Q dead),
    #              yT (big_4..7, K dead)
    big = ctx.enter_context(tc.tile_pool(name="big", bufs=1))
    proj = ctx.enter_context(tc.tile_pool(name="proj", bufs=1))
    # one PSUM pool for the whole kernel; 8 banks total:
    #   tp (transpose staging, f32-sized, bufs=2)       -> 2 banks
    #   pp (proj/outproj accum, f32 [128,512], bufs=1)  -> 1 bank
    #   s2 (scores, f32 [128,1024], bufs=2)             -> 4 banks
    #   o2 (PV accum, f32 [65,512], bufs=1)             -> 1 bank
    psum = ctx.enter_context(tc.tile_pool(name="psum", bufs=1, space="PSUM"))
    stage = ctx.enter_context(tc.tile_pool(name="stage", bufs=4))
    p2pool = ctx.enter_context(tc.tile_pool(name="p2pool", bufs=3))

    ident = const.tile([P, P], BF16, name="ident", tag="ident")
    make_identity(nc, ident)

    bo_sb = const.tile([P, ET], FP32, name="bo_sb", tag="bo_sb")
    dma(out=bo_sb, in_=dbo[:].rearrange("(t p) -> p t", p=P))

    l1 = const.tile([1, H * N], FP32, name="l1", tag="l1")
    ltmp = const.tile([P, 2 * N // P], FP32, name="ltmp", tag="ltmp")

    cast_flip = [0]

    def load_cast(dX, r):
        """DMA [128,E] f32 slice r, cast to bf16 (alternating DVE/ACT)."""
        x_f32 = stage.tile([P, E], FP32, name="x_f32", tag="x_f32", bufs=4)
        dma(out=x_f32, in_=dX[r * P:(r + 1) * P, :])
        x_bf = stage.tile([P, E], BF16, name="x_bf", tag="x_bf", bufs=6)
        cast_flip[0] ^= 1
        if cast_flip[0]:
            nc.vector.tensor_copy(x_bf, x_f32)
        else:
            nc.scalar.copy(x_bf, x_f32)
        return x_bf

    # ---- stage 0: weights ----
    wt = {}
    for wname, _ in (("q", dWq), ("k", dWk), ("v", dWv), ("o", dWo)):
        wt[wname] = [const.tile([P, E], BF16, name=f"w{wname}T_{c}",
                                tag=f"w{wname}T_{c}") for c in range(ET)]
    for wname, dW in (("q", dWq), ("k", dWk), ("v", dWv), ("o", dWo)):
        w_bfs = [load_cast(dW, r) for r in range(ET)]
        for c in range(ET):
            tp = psum.tile([P, E], BF16, name="tp", tag="tp", bufs=2)
            for r in range(ET):
                nc.tensor.transpose(
                    tp[:, r * P:(r + 1) * P], w_bfs[r][:, c * P:(c + 1) * P],
                    ident)
            nc.vector.tensor_copy(wt[wname][c], tp)

    # ---- stage 0: inputs -> X^T ----
    xT = {}
    slot = {"Q": 0, "K": 4, "V": 8}
    for xname, dX in (("Q", dQ), ("K", dK), ("V", dV)):
        xT[xname] = [big.tile([P, N], BF16, name=f"{xname}T_{et}",
                              tag=f"big_{slot[xname] + et}")
                     for et in range(ET)]
        for g in range(NT // ET):  # groups of 4 n-tiles
            x_bfs = [load_cast(dX, g * ET + i) for i in range(ET)]
            for et in range(ET):
                tp = psum.tile([P, E], BF16, name="tp", tag="tp", bufs=2)
                for i in range(ET):
                    nc.tensor.transpose(
                        tp[:, i * P:(i + 1) * P],
                        x_bfs[i][:, et * P:(et + 1) * P], ident)
                nc.vector.tensor_copy(
                    xT[xname][et][:, g * E:(g + 1) * E], tp)

    # ---- projections ----
    qT = [proj.tile([P, N], BF16, name=f"qT_{m}", tag=f"qT_{m}")
          for m in range(ET)]
    kT = [proj.tile([P, N], BF16, name=f"kT_{m}", tag=f"kT_{m}")
          for m in range(ET)]
    v_aug = [proj.tile([P, H * HV], BF16, name=f"vaug_{nt}",
                       tag=f"vaug_{nt}") for nt in range(NT)]

    def emit_qk_proj(m, c):
        """qT[m] and kT[m], n-chunk c: two 4-MM groups + drains."""
        for pname, outs, xtiles in (("q", qT, xT["Q"]), ("k", kT, xT["K"])):
            ps = psum.tile([P, 512], FP32, name="pp", tag="pp", bufs=1)
            for et in range(ET):
                nc.tensor.matmul(
                    ps,
                    lhsT=wt[pname][et][:, m * P:(m + 1) * P],
                    rhs=xtiles[et][:, c * 512:(c + 1) * 512],
                    start=(et == 0), stop=(et == ET - 1))
            nc.vector.tensor_copy(outs[m][:, c * 512:(c + 1) * 512], ps)

    def emit_v_proj(nt):
        ps = psum.tile([P, 512], FP32, name="pp", tag="pp", bufs=1)
        for et in range(ET):
            nc.tensor.matmul(
                ps,
                lhsT=xT["V"][et][:, nt * P:(nt + 1) * P],
                rhs=wt["v"][et],
                start=(et == 0), stop=(et == ET - 1))
        va = v_aug[nt].rearrange("p (h c) -> p h c", c=HV)
        nc.vector.tensor_copy(
            va[:, :, 0:D], ps.rearrange("p (h d) -> p h d", d=D))
        nc.gpsimd.memset(va[:, :, D:HV], 1.0)

    # v fully + head-pair 0 projections up front
    for nt in range(NT):
        emit_v_proj(nt)
    for c in range(ET):
        emit_qk_proj(0, c)

    # ---- attention, with interleaved proj filler ----
    oT = [big.tile([P, N], BF16, name=f"oT_{m}", tag=f"big_{8 + m}")
          for m in range(ET)]

    filler = deque()
    for hp in range(1, ET):
        for c in range(ET):
            filler.append((hp, c))

    def emit_pv(h, o2, p2, ktp):
        for j in range(2):
            kt = 2 * ktp + j
            nc.tensor.matmul(
                o2[:, :],
                lhsT=v_aug[kt][:, h * HV:(h + 1) * HV],
                rhs=p2[:, j * QC:(j + 1) * QC],
                start=(kt == 0), stop=(kt == NT - 1),
                skip_group_check=True)

    def attn_chunk(h, qc):
        hp, h2 = h // 2, (h % 2) * 64
        o2 = psum.tile([HV, QC], FP32, name="o2", tag="o2", bufs=1)
        rhs_q = qT[hp][h2:h2 + 64, qc * QC:(qc + 1) * QC]
        prev = None  # (p2, ktp) pending PV
        for ktp in range(KTP):
            s2 = psum.tile([P, 2 * QC], FP32, name="s2", tag="s2", bufs=2)
            for j in range(2):
                kt = 2 * ktp + j
                lhs_k = kT[hp][h2:h2 + 64, kt * P:(kt + 1) * P]
                nc.tensor.matmul(s2[:, j * QC:(j + 1) * QC], lhsT=lhs_k,
                                 rhs=rhs_q, start=True, stop=True)
            for _ in range(WARM_DUP):
                gb = psum.tile([P, QC], FP32, name="gb", tag="tp", bufs=2)
                nc.tensor.matmul(gb, lhsT=kT[hp][h2:h2 + 64, 0:P],
                                 rhs=rhs_q, start=True, stop=True)
            if prev is not None:
                emit_pv(h, o2, *prev)
            p2 = p2pool.tile([P, 2 * QC], BF16, name="p2", tag="p2")
            nc.scalar.activation(p2, s2, AF.Exp, scale=inv_tau)
            prev = (p2, ktp)
            if ktp % 2 == 1 and filler:
                emit_qk_proj(*filler.popleft())
        emit_pv(h, o2, *prev)
        nc.vector.tensor_copy(
            l1[0:1, h * N + qc * QC:h * N + (qc + 1) * QC], o2[D:HV, :])
        nc.vector.tensor_copy(
            oT[hp][h2:h2 + 64, qc * QC:(qc + 1) * QC], o2[0:D, :])

    oTn = [big.tile([P, N], BF16, name=f"oTn_{m}", tag=f"big_{m}")
           for m in range(ET)]

    def emit_norm(hp):
        # spread [1, 2N] row sums across 128 partitions, reciprocal, spread
        # back out to DRAM in the same linear order
        lsl = l1[0:1, 2 * hp * N:2 * (hp + 1) * N]
        dma(out=ltmp, in_=lsl)
        nc.vector.reciprocal(ltmp, ltmp)
        dma(out=drs[2 * hp * N:2 * (hp + 1) * N], in_=ltmp)
        for qc in range(NQC):
            rb = stage.tile([P, QC], FP32, name="rb", tag="rb", bufs=2)
            for h2 in range(2):
                h = hp * 2 + h2
                bsrc = bass.AP(
                    tensor=drs,
                    offset=h * N + qc * QC,
                    ap=[[0, 64], [1, QC]])
                dma(out=rb[h2 * 64:(h2 + 1) * 64, :], in_=bsrc)
            nc.vector.tensor_tensor(
                oTn[hp][:, qc * QC:(qc + 1) * QC],
                oT[hp][:, qc * QC:(qc + 1) * QC],
                rb, ALU.mult)

    for h in range(H):
        for qc in range(NQC):
            attn_chunk(h, qc)
        if h % 2 == 1:
            emit_norm(h // 2)

    # ---- output projection: Y^T = Wo @ O^T + bo ----
    yT = [big.tile([P, N], BF16, name=f"yT_{m}", tag=f"big_{4 + m}")
          for m in range(ET)]
    for m in range(ET):
        for c in range(ET):
            ps = psum.tile([P, 512], FP32, name="ps_o", tag="s2", bufs=2)
            for et in range(ET):
                nc.tensor.matmul(
                    ps,
                    lhsT=wt["o"][et][:, m * P:(m + 1) * P],
                    rhs=oTn[et][:, c * 512:(c + 1) * 512],
                    start=(et == 0), stop=(et == ET - 1))
            nc.scalar.add(yT[m][:, c * 512:(c + 1) * 512], ps,
                          bo_sb[:, m:m + 1])

    # ---- transpose back + store ----
    for nt in range(NT):
        tp = psum.tile([P, E], BF16, name="tp", tag="tp", bufs=2)
        for m in range(ET):
            nc.tensor.transpose(
                tp[:, m * P:(m + 1) * P],
                yT[m][:, nt * P:(nt + 1) * P], ident)
        y_sb = stage.tile([P, E], FP32, name="y_sb", tag="y_sb", bufs=2)
        nc.scalar.copy(y_sb, tp)
        dma(out=dout[nt * P:(nt + 1) * P, :], in_=y_sb)


_CACHE = {}


def _get_nc(inv_tau: float) -> bass.Bass:
    key = round(float(inv_tau), 9)
    if key not in _CACHE:
        _CACHE[key] = _build(float(inv_tau))
    return _CACHE[key]


def _run(inputs: dict, trace: bool = False):
    """Returns (output [B,N,E] fp32, BassKernelResults)."""
    from concourse.bass_utils import run_bass_kernel_spmd

    Q = np.ascontiguousarray(np.asarray(inputs["Q"], dtype=np.float32))
    K = np.ascontiguousarray(np.asarray(inputs["K"], dtype=np.float32))
    V = np.ascontiguousarray(np.asarray(inputs["V"], dtype=np.float32))
    Wq = np.ascontiguousarray(np.asarray(inputs["Wq"], dtype=np.float32))
    Wk = np.ascontiguousarray(np.asarray(inputs["Wk"], dtype=np.float32))
    Wv = np.ascontiguousarray(np.asarray(inputs["Wv"], dtype=np.float32))
    Wo = np.ascontiguousarray(np.asarray(inputs["Wo"], dtype=np.float32))
    bo = np.ascontiguousarray(np.asarray(inputs["bo"], dtype=np.float32))
    tau = float(np.asarray(inputs["tau"]))

    mask = inputs.get("attn_mask")
    if mask is not None and not np.all(np.asarray(mask) != 0):
        # Fallback (never hit for the spec'd all-ones mask): host math.
        return _host_reference(Q, K, V, np.asarray(mask), Wq, Wk, Wv, Wo,
                               bo, tau), None

    nc = _get_nc(1.0 / tau)
    in_maps = []
    for b in range(NCORES):
        in_maps.append({
            "Q": Q[b], "K": K[b], "V": V[b],
            "Wq": Wq, "Wk": Wk, "Wv": Wv, "Wo": Wo, "bo": bo,
        })
    res = run_bass_kernel_spmd(nc, in_maps, list(range(NCORES)), trace=trace)
    out = np.stack([np.asarray(res.results[b]["out"]) for b in range(NCORES)])
    return out.astype(np.float32), res


def _host_reference(Q, K, V, mask, Wq, Wk, Wv, Wo, bo, tau):
    b, n, _ = Q.shape
    q = (Q @ Wq.T).reshape(b, n, H, D).transpose(0, 2, 1, 3)
    k = (K @ Wk.T).reshape(b, n, H, D).transpose(0, 2, 1, 3)
    v = (V @ Wv.T).reshape(b, n, H, D).transpose(0, 2, 1, 3)
    s = np.einsum("bhnd,bhmd->bhnm", q, k) / tau
    s = np.where(mask == 0, -np.inf, s)
    s = s - s.max(axis=-1, keepdims=True)
    e = np.exp(s)
    a = e / e.sum(axis=-1, keepdims=True)
    o = np.einsum("bhnm,bhmd->bhnd", a, v)
    o = o.transpose(0, 2, 1, 3).reshape(b, n, H * D)
    return (o @ Wo.T + bo).astype(np.float32)


def kernel(**inputs) -> np.ndarray:
    out, _ = _run(inputs, trace=False)
    return out
